# revision 31
# baseline (speedup 1.0000x reference)
"""Distributed spectral conv on S2 (SHT -> per-l complex channel mix -> ISHT)
for Trainium2, m-mode sharded across 8 NeuronCores.

Pipeline per core (33 of 257 rfft m-modes per core, zero-padded):
  A: DFT over lon, basis-as-weights fp32r N=512   -> psum [66cm, 512ck]
  T1: PE-transpose pivot                          -> XFT [k, (comp,m,c)] f32r
  B: Legendre transform fp32r (contract k)        -> CFQ1/CFQ2 [(s,i), (j,cm)] bf16
  C: per-l-pair channel mix, block-diag bf16      -> COUT4 [o, (l,cm)] bf16
  P1: PE-transpose pivot                          -> OUTT [l, (m,comp,o)] bf16
  D: inverse Legendre bf16 (contract l)           -> XKS [k', (m,comp,o)] bf16
  P2: PE-transpose pivot                          -> XK [(m,comp), (o,k')] bf16
  E: inverse DFT bf16 (contract m-comps)          -> y_part [(o,k'), n] f32
Host sums the 8 partial y outputs (linear in m-modes).
"""
import numpy as np
import ml_dtypes

import concourse.bass as bass
import concourse.bacc as bacc
import concourse.mybir as mybir
from concourse import tile
from concourse._compat import get_trn_type
from concourse.bass_utils import run_bass_kernel_spmd

F32 = mybir.dt.float32
F32R = mybir.dt.float32r
BF16 = mybir.dt.bfloat16

N_CORES = 8
M_LOC = 33            # m modes per core (8*33 = 264 >= 257, rest zero-padded)
MC = 2 * M_LOC        # real+imag components
CIN = 64
COUT_ = 64
NLAT = 256
NLON = 512
MMAX = 257
CK = COUT_ * NLAT     # 16384 output rows
WCHUNK = 16           # j-pairs per weight DMA chunk

_prog_cache = {}


def _build_nc(stages="ABCDE"):
    nc = bacc.Bacc(get_trn_type() or "TRN2", target_bir_lowering=False, debug=False)

    xt = nc.dram_tensor("xt", [4, 128, CK], BF16, kind="ExternalInput")
    fdft = nc.dram_tensor("fdft", [4, 128, MC], BF16, kind="ExternalInput")
    shtw_t = nc.dram_tensor("shtw_t", [M_LOC, 2, 128, 256], F32R, kind="ExternalInput")
    wblk = nc.dram_tensor("wblk", [128, 128, 3, 128], BF16, kind="ExternalInput")
    pct_t = nc.dram_tensor("pct_t", [M_LOC, 2, 128, 256], BF16, kind="ExternalInput")
    gdft = nc.dram_tensor("gdft", [MC, NLON], BF16, kind="ExternalInput")
    ident = nc.dram_tensor("ident", [128, 128], F32, kind="ExternalInput")
    y_part = nc.dram_tensor("y_part", [CK, NLON], BF16, kind="ExternalOutput")

    with tile.TileContext(nc) as tc:
        with tc.tile_pool(name="const", bufs=1) as constp, \
             tc.tile_pool(name="big", bufs=1) as bigp, \
             tc.tile_pool(name="xa", bufs=4) as xap, \
             tc.tile_pool(name="xf2", bufs=6) as xf2p, \
             tc.tile_pool(name="sw", bufs=8) as swp, \
             tc.tile_pool(name="wt", bufs=2) as wtp, \
             tc.tile_pool(name="pt", bufs=6) as ptp, \
             tc.tile_pool(name="ysb", bufs=8) as ysbp, \
             tc.tile_pool(name="ps", bufs=4, space="PSUM") as psp, \
             tc.tile_pool(name="pst", bufs=4, space="PSUM") as pstp:

            xar0 = xap.tile([128, 4, 1024], BF16, tag="xar")
            nc.sync.dma_start(
                xar0[:, :, :],
                xt.ap()[:, :, 0:1024].rearrange("a b c -> b a c")
            )
            fsbr = constp.tile([128, 4, MC], BF16)      # [n_in_chunk, nchunk, cm]
            gsb = constp.tile([MC, NLON], BF16)
            isb = constp.tile([128, 128], F32)
            isbb = constp.tile([128, 128], BF16)
            nc.sync.dma_start(fsbr[:, :, :], fdft.ap().rearrange("a b c -> b a c"))
            nc.sync.dma_start(gsb[:, :], gdft[:, :])
            nc.sync.dma_start(isb[:, :], ident[:, :])
            nc.vector.tensor_copy(isbb[:, :], isb[:, :])

            # ---- stage A: DFT as matmul, basis stationary, fp32r ----
            # then T1 transposes into XFT[ki, kh*4224 + cm*64 + c]
            XFT = bigp.tile([128, 2 * MC * 64], F32R, tag="bigA")
            XFT_v = XFT.rearrange("p (kh mm comp c) -> p kh mm comp c",
                                  kh=2, comp=2, c=64)
            for span in range(16):          # 1024 ck-columns per span
                if span == 0:
                    xar = xar0
                else:
                    xar = xap.tile([128, 4, 1024], BF16, tag="xar")
                    nc.sync.dma_start(
                        xar[:, :, :],
                        xt.ap()[:, :, span * 1024:(span + 1) * 1024]
                        .rearrange("a b c -> b a c")
                    )
                for sub in range(2):
                    c = span * 4 + sub * 2  # chunk covers channels c, c+1
                    pa = psp.tile([MC, 512], F32, tag="ps")
                    for nc4 in range(4):
                        nc.tensor.matmul(
                            pa[:, :],
                            fsbr[:, nc4, :],
                            xar[:, nc4, sub * 512:(sub + 1) * 512],
                            start=(nc4 == 0),
                            stop=(nc4 == 3),
                        )
                    xf2 = xf2p.tile([MC, 512], F32)
                    for q in range(4):
                        if (sub + q) % 2 == 0:
                            nc.vector.tensor_copy(
                                xf2[:, q * 128:(q + 1) * 128], pa[:, q * 128:(q + 1) * 128])
                        else:
                            nc.scalar.copy(
                                xf2[:, q * 128:(q + 1) * 128], pa[:, q * 128:(q + 1) * 128])
                    for q in range(4):
                        cc = c + q // 2
                        kh = q % 2
                        ptr = pstp.tile([128, MC], F32, tag="pst")
                        nc.tensor.transpose(
                            ptr[:, :], xf2[:, q * 128:(q + 1) * 128], isb[:MC, :MC]
                        )
                        dstx = XFT_v[:, kh, :, :, cc].rearrange("p mm comp -> p comp mm")
                        srcx = ptr.rearrange("p (comp mm) -> p comp mm", comp=2)
                        if q % 2 == 0:
                            nc.vector.tensor_copy(dstx, srcx)
                        else:
                            nc.scalar.copy(dstx, srcx)

            if "B" not in stages:
                dbg = ysbp.tile([128, NLON], BF16, tag="ys")
                nc.vector.tensor_copy(dbg[:, :], XFT[:, 0:NLON].bitcast(F32))
                nc.sync.dma_start(y_part[0:128, :], dbg[:, :])
                return nc
            # ---- stage B: Legendre fp32r -> block-layout bf16 coeffs ----
            # CFQ[(s,i), (cm, j)] = cf_comp[i, l=2j+s, m] with cm=(comp,m)
            CFQ = bigp.tile([128, MC * 128], BF16, tag="bigB")
            CFQ_v = CFQ.rearrange("p (cm j) -> p cm j", j=128)
            for m in range(M_LOC):
                swr = swp.tile([128, 2, 256], F32R)     # [ki, kh, l]
                nc.sync.dma_start(
                    swr[:, :, :], shtw_t[m].rearrange("a b c -> b a c")
                )
                # both comps in one M=128 matmul: out rows (comp, i)
                pb = psp.tile([128, 256], F32, tag="ps")
                for kh in range(2):
                    nc.tensor.matmul(
                        pb[:, :],
                        XFT.rearrange("p (kh mm cc) -> p kh mm cc", kh=2, cc=128)[:, kh, m, :],
                        swr[:, kh, :],
                        start=(kh == 0),
                        stop=(kh == 1),
                    )
                for comp in range(2):
                    for si in range(2):
                        dst = CFQ_v[si * 64:(si + 1) * 64, comp * M_LOC + m, :]
                        src = pb[comp * 64:(comp + 1) * 64,
                                 si * 128:(si + 1) * 128]
                        if (m + si) % 2 == 0:
                            nc.vector.tensor_copy(dst, src)
                        else:
                            nc.scalar.copy(dst, src)

            if "C" not in stages:
                dbg = ysbp.tile([128, NLON], BF16, tag="ys")
                nc.vector.tensor_copy(dbg[:, :], CFQ[:, 0:NLON])
                nc.sync.dma_start(y_part[0:128, :], dbg[:, :])
                return nc
            # ---- stage C: channel mix, block-diag per l-pair, bf16 (3 MMs) ----
            # rhs slices of CFQ: cfr cols = CFQ[:, 0:33, j], cfi cols = CFQ[:, 33:66, j]
            # out[:, 0:33] = Wr.T cfr - Wi.T cfi ; out[:, 33:66] = Wr.T cfi + Wi.T cfr
            # COUT4[o, cm*256 + l]
            COUT4 = bigp.tile([64, MC * 256], BF16, tag="bigA")
            COUT4_v = COUT4.rearrange("p (cm l) -> p cm l", l=256)
            for ci in range(128 // WCHUNK):
                wt = wtp.tile([128, WCHUNK, 3, 128], BF16)
                nc.gpsimd.dma_start(
                    wt[:, :, :, :],
                    wblk.ap()[:, ci * WCHUNK:(ci + 1) * WCHUNK, :, :]
                )
                for jj in range(0, WCHUNK, 2):
                    j = ci * WCHUNK + jj
                    # two l-pairs (j, j+1) share one psum tile: cols [0:66|66:132]
                    pc = psp.tile([128, 2 * MC], F32, tag="ps")
                    pcv = pc.rearrange("p (h cm) -> p h cm", h=2)
                    for h in range(2):
                        first = (h == 0)
                        last = (h == 1)
                        # v0 = wr (full 66), v1 = wi, v2 = -wi
                        nc.tensor.matmul(pcv[:, h, :], wt[:, jj + h, 0, :],
                                         CFQ_v[:, :, j + h],
                                         start=first, stop=False)
                        nc.tensor.matmul(pcv[:, h, 0:M_LOC], wt[:, jj + h, 2, :],
                                         CFQ_v[:, M_LOC:MC, j + h],
                                         start=False, stop=False)
                        nc.tensor.matmul(pcv[:, h, M_LOC:MC], wt[:, jj + h, 1, :],
                                         CFQ_v[:, 0:M_LOC, j + h],
                                         start=False, stop=last)
                    d0 = COUT4_v[:, :, j:j + 2].rearrange("p cm h -> p h cm")
                    d1 = COUT4_v[:, :, 128 + j:130 + j].rearrange("p cm h -> p h cm")
                    if (j // 2) % 2 == 0:
                        nc.vector.tensor_copy(d0, pcv[0:64, :, :])
                        nc.scalar.copy(d1, pcv[64:128, :, :])
                    else:
                        nc.scalar.copy(d0, pcv[0:64, :, :])
                        nc.vector.tensor_copy(d1, pcv[64:128, :, :])

            if "P1" not in stages and "D" not in stages:
                dbg = ysbp.tile([64, NLON], BF16, tag="ys")
                nc.vector.tensor_copy(dbg[:, :], COUT4[:, 0:NLON])
                nc.sync.dma_start(y_part[0:64, :], dbg[:, :])
                return nc
            # ---- pivot P1: COUT4 -> OUTT[l, (m,comp,o)] via PE transpose ----
            OUTT = bigp.tile([128, 2 * MC * 64], BF16, tag="bigC")
            OUTT_v = OUTT.rearrange("p (lc m cp o) -> p lc m cp o", lc=2, cp=2, o=64)
            for cm in range(MC):
                cp, m = cm // M_LOC, cm % M_LOC
                for lc in range(2):
                    ptr = pstp.tile([128, 64], BF16, tag="pst")
                    nc.tensor.transpose(
                        ptr[:, :],
                        COUT4_v[:, cm, lc * 128:(lc + 1) * 128], isbb[:64, :64]
                    )
                    nc.vector.tensor_copy(OUTT_v[:, lc, m, cp, :], ptr[:, :])

            if "D" not in stages:
                dbg = ysbp.tile([128, NLON], BF16, tag="ys")
                nc.vector.tensor_copy(dbg[:, :], OUTT[:, 0:NLON])
                nc.sync.dma_start(y_part[0:128, :], dbg[:, :])
                return nc
            # ---- stage D: inverse Legendre bf16, contract l ----
            # XKS[ki', kc*8192 + o*128 + (m*2+cp)]  (cm contiguous, padded to 128)
            XKS = bigp.tile([128, 2 * 64 * 128], BF16, tag="bigA")
            XKS_v = XKS.rearrange("p (kc o cm) -> p kc o cm", kc=2, o=64)
            for m in range(M_LOC):
                pt = ptp.tile([128, 2, 256], BF16)      # [li, lc, kp]
                nc.sync.dma_start(
                    pt[:, :, :], pct_t[m].rearrange("a b c -> b a c")
                )
                for kc in range(2):
                    pd = psp.tile([128, 128], F32, tag="ps")
                    for lc in range(2):
                        nc.tensor.matmul(
                            pd[:, :],
                            pt[:, lc, kc * 128:(kc + 1) * 128],
                            OUTT_v[:, lc, m, :, :],
                            start=(lc == 0),
                            stop=(lc == 1),
                        )
                    # psum cols (cp, o) -> dest (o, cm=m*2+cp): cp outer step 1, o inner step 66
                    dv = XKS_v[:, kc, :, 2 * m:2 * m + 2].rearrange("p o c -> p c o")
                    if m % 2 == 0:
                        nc.vector.tensor_copy(dv, pd.rearrange("p (c o) -> p c o", c=2))
                    else:
                        nc.scalar.copy(dv, pd.rearrange("p (c o) -> p c o", c=2))

            if "P2" not in stages and "E" not in stages:
                dbg = ysbp.tile([128, NLON], BF16, tag="ys")
                nc.vector.tensor_copy(dbg[:, :], XKS[:, 0:NLON])
                nc.sync.dma_start(y_part[0:128, :], dbg[:, :])
                return nc
            # ---- pivot P2: XKS -> XK[(m,comp), (o,k')] via PE transpose ----
            XK = bigp.tile([MC, CK], BF16, tag="bigB")
            XK_v = XK.rearrange("p (o k) -> p o k", k=NLAT)
            for o in range(64):
                for kc in range(2):
                    pt2 = pstp.tile([MC, 128], BF16, tag="pst")
                    nc.tensor.transpose(
                        pt2[:, :], XKS_v[:, kc, o, 0:MC], isbb[:, :]
                    )
                    nc.vector.tensor_copy(
                        XK_v[:, o, kc * 128:(kc + 1) * 128], pt2[:, :]
                    )

            if "E" not in stages:
                dbg = ysbp.tile([MC, NLON], BF16, tag="ys")
                nc.vector.tensor_copy(dbg[:, :], XK[0:MC, 0:NLON])
                nc.sync.dma_start(y_part[0:MC, :], dbg[:, :])
                return nc
            # ---- stage E: inverse DFT as matmul bf16, contract m-comps ----
            for jp in range(CK // 256):
                ys = ysbp.tile([128, 2, NLON], BF16, tag="ys2")
                for h in range(2):
                    j = 2 * jp + h
                    pe = psp.tile([128, NLON], F32, tag="ps")
                    nc.tensor.matmul(
                        pe[:, :], XK[:, j * 128:(j + 1) * 128], gsb[:, :],
                        start=True, stop=True,
                    )
                    if h == 0:
                        nc.vector.tensor_copy(ys[:, h, :], pe[:, :])
                    else:
                        nc.scalar.copy(ys[:, h, :], pe[:, :])
                nc.gpsimd.dma_start(
                    y_part.ap()[jp * 256:(jp + 1) * 256, :]
                    .rearrange("(a p) n -> p a n", a=2),
                    ys[:, :, :],
                )

    return nc


def _get_nc(stages="ABCDE"):
    if stages not in _prog_cache:
        nc = _build_nc(stages)
        nc.compile()
        _prog_cache[stages] = nc
    return _prog_cache[stages]


def _core_ms(r):
    return [r * M_LOC + j for j in range(M_LOC) if r * M_LOC + j < MMAX]


def make_in_maps(x, weight_r, weight_i, pct, sht_w):
    x = np.asarray(x, dtype=np.float32)
    wr = np.asarray(weight_r, dtype=np.float32)[0]          # [i, o, l]
    wi = np.asarray(weight_i, dtype=np.float32)[0]
    pct = np.asarray(pct, dtype=np.float32)                 # [m, l, k]
    sht_w = np.asarray(sht_w, dtype=np.float32)

    # xt[nc4, ni, ck] with n = nc4*128 + ni, ck = c*256 + k
    xt = np.ascontiguousarray(
        x[0].reshape(CK, NLON).T.reshape(4, 128, CK)
    ).astype(ml_dtypes.bfloat16)
    # wblk[(s,i), j, v, (s,o)]: block-diag of w_v[:, :, 2j+s], v0=wr v1=wi v2=-wi
    wb = np.zeros((128, 3, 128, 128), np.float32)
    wb[:, 0, 0:64, 0:64] = wr[:, :, 0:128].transpose(2, 0, 1)
    wb[:, 0, 64:128, 64:128] = wr[:, :, 128:256].transpose(2, 0, 1)
    wb[:, 1, 0:64, 0:64] = wi[:, :, 0:128].transpose(2, 0, 1)
    wb[:, 1, 64:128, 64:128] = wi[:, :, 128:256].transpose(2, 0, 1)
    wb[:, 2] = -wb[:, 1]
    wblk = np.ascontiguousarray(wb.transpose(2, 0, 1, 3)).astype(ml_dtypes.bfloat16)
    ident = np.eye(128, dtype=np.float32)

    n = np.arange(NLON)
    in_maps = []
    for r in range(N_CORES):
        ms = _core_ms(r)
        nm = len(ms)
        marr = np.array(ms)

        ang = 2.0 * np.pi * marr[None, :] * n[:, None] / NLON   # [n, nm]
        fdft = np.zeros((NLON, MC), np.float32)
        fdft[:, :nm] = (2.0 * np.pi / NLON) * np.cos(ang)
        fdft[:, M_LOC:M_LOC + nm] = -(2.0 * np.pi / NLON) * np.sin(ang)
        fdft = fdft.reshape(4, 128, MC)

        cmf = np.where((marr == 0) | (marr == NLON // 2), 1.0, 2.0)
        # gdft rows INTERLEAVED (m, comp): row 2j = c cos, row 2j+1 = -c sin
        gdft = np.zeros((MC, NLON), np.float32)
        gdft[0:2 * nm:2, :] = cmf[:, None] * np.cos(ang.T)
        gdft[1:2 * nm:2, :] = -cmf[:, None] * np.sin(ang.T)

        shtw_t = np.zeros((M_LOC, 2, 128, 256), np.float32)
        shtw_t[:nm] = sht_w[marr].transpose(0, 2, 1).reshape(nm, 2, 128, 256)

        pct_t = np.zeros((M_LOC, 2, 128, 256), np.float32)
        pct_t[:nm] = pct[marr].reshape(nm, 2, 128, 256)

        in_maps.append({
            "xt": xt, "fdft": np.ascontiguousarray(fdft).astype(ml_dtypes.bfloat16),
            "shtw_t": shtw_t, "wblk": wblk,
            "pct_t": pct_t.astype(ml_dtypes.bfloat16),
            "gdft": gdft.astype(ml_dtypes.bfloat16), "ident": ident,
        })
    return in_maps


def kernel(x, weight_r, weight_i, pct, sht_w):
    x_np = np.asarray(x)
    nc = _get_nc()
    in_maps = make_in_maps(x_np, weight_r, weight_i, pct, sht_w)
    try:
        res = run_bass_kernel_spmd(nc, in_maps, list(range(N_CORES)))
    except Exception:
        # transient NRT exec faults have been observed on the first run
        # after a NEFF load; one retry has always succeeded
        res = run_bass_kernel_spmd(nc, in_maps, list(range(N_CORES)))
    y = np.zeros((CK, NLON), np.float64)
    for r in range(N_CORES):
        y += np.asarray(res.results[r]["y_part"], dtype=np.float64)
    y = y.astype(np.float32).reshape(1, COUT_, NLAT, NLON)
    return (y, x_np)
